# revision 1
# baseline (speedup 1.0000x reference)
"""BinaryTreeCRF inside-algorithm kernel for 8 Trainium2 NeuronCores.

Strategy (hardcoded for hidden=[16383,1024], L=32, depth 13):
  - The 16383-node heap tree is cut at big-tree level 3: each of the 8 cores
    owns the 2047-node subtree rooted at heap node 7+c (big levels 3..13).
  - Per core, node hidden states are shipped transposed, with tree levels
    bit-reversed so left/right children are contiguous half-blocks, and the
    columns grouped into pass-major blocks so each combine pass depends on
    exactly one HBM block: compute chases the (staggered) block loads.
  - On device (bf16, label-on-partition layout):
      E^T = W @ hsT + b            (PE, 8 K-chunks, per column-block)
      two combine levels (1024 leaves -> 512 -> 256 nodes) using the
      residual/accumulator decomposition  score = resid + acc:
        logP[(l,r), j] = (resid_l[l,j]-mean_l[j]) + (resid_r[r,j]-mean_r[j])
                         via selector matmuls with the mean folded in,
        P = exp(logP)  (ACT),  S^T = Texp @ P  (PE),
        resid' = E^T + ln S^T (ACT+DVE), acc' = acc_l+acc_r+mean_l+mean_r.
  - Host finishes the remaining small levels (256 -> subtree roots -> root)
    in float64 numpy; this is ~1% of the FLOPs.
"""

import numpy as np
import ml_dtypes

BF16 = ml_dtypes.bfloat16

INPUT_SIZE = 1024
L = 32
DEPTH = 13
N_CORES = 8
SUB_LEVELS = 11       # per-core subtree levels: 0 = 1024 leaves ... 10 = root
COLS = 2048           # per-core columns (2047 nodes + 1 zero pad)

# "old" layout: levels from the leaves up, each level bit-reversed.
OFFS = []
_o = 0
for _l in range(SUB_LEVELS):
    OFFS.append(_o)
    _o += 1 << (10 - _l)
assert _o == 2047

# "new" (pass-major) layout: two 768-col blocks [rl | rr | elev] for the two
# level-1 passes (256 parents each), then the level-2 elev block + host tail.
BLOCK_SIZES = [768, 768, 256, 256]
BLOCK_STARTS = np.concatenate([[0], np.cumsum(BLOCK_SIZES)])[:-1]
NEWCOL_TO_OLD = np.empty(COLS, dtype=np.int64)
for _g in range(2):
    _b = _g * 768
    NEWCOL_TO_OLD[_b:_b + 256] = np.arange(_g * 256, _g * 256 + 256)
    NEWCOL_TO_OLD[_b + 256:_b + 512] = 512 + np.arange(_g * 256, _g * 256 + 256)
    NEWCOL_TO_OLD[_b + 512:_b + 768] = 1024 + np.arange(_g * 256, _g * 256 + 256)
NEWCOL_TO_OLD[1536:COLS] = np.arange(1536, COLS)


def _bitrev(x, bits):
    x = np.asarray(x, dtype=np.int64)
    out = np.zeros_like(x)
    for i in range(bits):
        out = (out << 1) | ((x >> i) & 1)
    return out


def _core_col_heap_index(c):
    """heap index for each of the 2047 real old-layout columns of core c."""
    idx = np.zeros(2047, dtype=np.int64)
    for lev in range(SUB_LEVELS):
        m = 1 << (10 - lev)
        d = DEPTH - lev
        q = np.arange(m)
        j = _bitrev(q, 10 - lev)
        idx[OFFS[lev]: OFFS[lev] + m] = (1 << d) - 1 + c * m + j
    return idx


def _selectors():
    """Selector matrices (mean-subtraction folded in) for the logP matmuls."""
    selL = np.full((L, 8 * 128), -1.0 / L, dtype=np.float32)
    selR = np.full((L, 128), -1.0 / L, dtype=np.float32)
    for ch in range(8):
        for p in range(128):
            selL[ch * 4 + p // 32, ch * 128 + p] += 1.0
    for p in range(128):
        selR[p % 32, p] += 1.0
    return selL.astype(BF16), selR.astype(BF16)


_NC = None


def _build_bass():
    global _NC
    if _NC is not None:
        return _NC
    from concourse import bacc, mybir
    from concourse.tile import TileContext
    from concourse.tile_rust import add_dep_helper

    dtb = mybir.dt.bfloat16
    dtf = mybir.dt.float32
    AF = mybir.ActivationFunctionType

    nc = bacc.Bacc()
    hsB = [nc.dram_tensor(f"hsB{g}", [1024, BLOCK_SIZES[g]], dtb,
                          kind="ExternalInput") for g in range(4)]
    # all constants in ONE tensor -> ONE DMA -> one HWDGE-lane dependency
    cpk = nc.dram_tensor("cpack", [128, 1680], dtb, kind="ExternalInput")
    outE = nc.dram_tensor("outE", [L, COLS], dtb, kind="ExternalOutput")
    outResid = nc.dram_tensor("outResid", [L, 256], dtb, kind="ExternalOutput")
    outAcc = nc.dram_tensor("outAcc", [1, 256], dtf, kind="ExternalOutput")

    with TileContext(nc) as tc:
        with tc.tile_pool(name="consts", bufs=1) as consts, \
             tc.tile_pool(name="hs", bufs=1) as hpool, \
             tc.tile_pool(name="state", bufs=1) as state, \
             tc.tile_pool(name="pbuf", bufs=2) as pbuf, \
             tc.tile_pool(name="tmp", bufs=4) as tmp, \
             tc.tile_pool(name="ps2", bufs=3, space="PSUM") as ps2, \
             tc.tile_pool(name="smps", bufs=2, space="PSUM") as smps:

            cp = consts.tile([128, 1680], dtb, tag="cpack")
            cp_dma = nc.sync.dma_start(out=cp, in_=cpk[:, :])
            cp_dma = cp_dma.ins if hasattr(cp_dma, "ins") else cp_dma
            wTr_t = cp[:, 0:256]
            texp_t = cp[:, 256:512]
            selL_t = cp[0:L, 512:1536]
            selR_t = cp[0:L, 1536:1664]
            onesM_t = cp[0:L, 1664:1665]
            bias_t = cp[0:L, 1665:1666]

            # Upcast bias to f32 (tensor_scalar_add needs an f32 scalar AP);
            # doubles as an ACT warm-up that absorbs the const-DMA wait.
            bias_f = tmp.tile([L, 1], dtf, tag="bias_f")
            nc.scalar.activation(out=bias_f, in_=bias_t, func=AF.Identity)

            # PE warm-up: junk matmuls on the const tile keep the PE HAM busy
            # through the load phase so real matmuls run at 2.4 GHz.
            scratch = smps.tile([1, 512], dtf, tag="small")
            for _ in range(10):
                nc.tensor.matmul(scratch, lhsT=onesM_t, rhs=cp[0:L, 0:512],
                                 start=True, stop=True)

            # hidden shard, all 8 K-chunks: [128, chunk, col]
            hs_all = hpool.tile([128, 8, COLS], dtb, tag="hs")
            dma_insts = []
            for g in range(4):
                s = int(BLOCK_STARTS[g])
                eng = nc.sync if g % 2 == 0 else nc.scalar
                di = eng.dma_start(
                    out=hs_all[:, :, s:s + BLOCK_SIZES[g]],
                    in_=hsB[g][:, :].rearrange("(c p) n -> p c n", p=128))
                di = di.ins if hasattr(di, "ins") else di
                dma_insts.append(di)
                # cpack (tiny) must land first: everything reads it, and
                # unconstrained it loses the HBM round-robin to the blocks.
                add_dep_helper(di, cp_dma, reason="consts first")
                if g >= 2:
                    add_dep_helper(dma_insts[g], dma_insts[g - 2],
                                   reason="stagger hbm load")

            E_bf = state.tile([L, COLS], dtb, tag="E_bf")
            resid1 = state.tile([L, 512], dtb, tag="resid1")
            acc1 = state.tile([1, 512], dtf, tag="acc1")
            resid2 = state.tile([L, 256], dtb, tag="resid2")
            acc2 = state.tile([1, 256], dtf, tag="acc2")

            def emit_E_block(g):
                s = int(BLOCK_STARTS[g])
                n = BLOCK_SIZES[g]
                psE = ps2.tile([L, n], dtf, tag="ps")
                nbs = [(0, min(n, 512))] + ([(512, n - 512)] if n > 512 else [])
                for c in range(8):
                    for o, w in nbs:
                        nc.tensor.matmul(
                            psE[:, o:o + w], lhsT=wTr_t[:, c * L:(c + 1) * L],
                            rhs=hs_all[:, c, s + o:s + o + w],
                            start=(c == 0), stop=(c == 7))
                nc.vector.tensor_scalar_add(out=E_bf[:, s:s + n], in0=psE,
                                            scalar1=bias_f)

            def combine_pass(rl, rr, elev, r_out, nj):
                """One combine sub-pass over nj=256 parents; returns mean psum.

                logP is split into two 4-chunk PSUM tiles (2 banks each) so
                the 8-bank PSUM budget allows cross-pass double buffering."""
                logPa = ps2.tile([128, 4, nj], dtf, tag="ps")
                logPb = ps2.tile([128, 4, nj], dtf, tag="ps")
                halves = [logPa, logPb]
                for c in range(8):
                    lp = halves[c // 4][:, c % 4, :]
                    nc.tensor.matmul(
                        lp, lhsT=selL_t[:, c * 128:(c + 1) * 128],
                        rhs=rl, start=True, stop=False)
                    nc.tensor.matmul(
                        lp, lhsT=selR_t,
                        rhs=rr, start=False, stop=True)
                # meansum = mean_l + mean_r, summed on the PE via accumulation
                mean = smps.tile([1, nj], dtf, tag="small")
                nc.tensor.matmul(mean, lhsT=onesM_t, rhs=rl,
                                 start=True, stop=False)
                nc.tensor.matmul(mean, lhsT=onesM_t, rhs=rr,
                                 start=False, stop=True)
                P = pbuf.tile([128, 8, nj], dtb, tag="P")
                nc.scalar.activation(out=P[:, 0:4, :], in_=logPa, func=AF.Exp)
                nc.scalar.activation(out=P[:, 4:8, :], in_=logPb, func=AF.Exp)
                S = smps.tile([L, nj], dtf, tag="small")
                for c in range(8):
                    nc.tensor.matmul(
                        S, lhsT=texp_t[:, c * L:(c + 1) * L], rhs=P[:, c, :],
                        start=(c == 0), stop=(c == 7))
                lnS = tmp.tile([L, nj], dtb, tag="lnS")
                nc.scalar.activation(out=lnS, in_=S, func=AF.Ln)
                nc.vector.tensor_add(r_out, lnS, elev)
                return mean

            # level-1 passes chase their blocks
            for g in range(2):
                emit_E_block(g)
                b = g * 768
                mean = combine_pass(
                    rl=E_bf[:, b:b + 256],
                    rr=E_bf[:, b + 256:b + 512],
                    elev=E_bf[:, b + 512:b + 768],
                    r_out=resid1[:, g * 256:(g + 1) * 256], nj=256)
                nc.vector.tensor_copy(acc1[:, g * 256:(g + 1) * 256], mean)

            emit_E_block(2)   # level-2 elev columns
            emit_E_block(3)   # host-tail columns

            # level 2: 512 -> 256, one pass of 256 parents
            mean = combine_pass(
                rl=resid1[:, 0:256],
                rr=resid1[:, 256:512],
                elev=E_bf[:, 1536:1792],
                r_out=resid2, nj=256)
            usum = tmp.tile([1, 256], dtf, tag="usum")
            nc.vector.tensor_add(usum, acc1[:, 0:256], acc1[:, 256:512])
            nc.vector.tensor_add(acc2, usum, mean)

            nc.gpsimd.dma_start(out=outE[:, :], in_=E_bf)
            nc.gpsimd.dma_start(out=outResid[:, :], in_=resid2)
            nc.gpsimd.dma_start(out=outAcc[:, :], in_=acc2)

    # Pin Exp/Ln/Identity to the one table set containing all three, so the
    # ACT engine loads its function table exactly once (the default picker
    # chooses per-function sets and reloads ~2.7us on every Exp<->Ln switch).
    import concourse.bacc as _bacc_mod
    from concourse.hw_specs import get_activation_tables as _gat
    _keep = "natural_log_exp_and_others"
    _pin = {AF.Exp, AF.Ln, AF.Identity, AF.Copy}

    def _gat_pinned(arch):
        t = _gat(arch)
        return {name: (funcs if name == _keep else (set(funcs) - _pin))
                for name, funcs in t.items()}

    _orig_gat = _bacc_mod.get_activation_tables
    _bacc_mod.get_activation_tables = _gat_pinned
    try:
        nc.compile()
    finally:
        _bacc_mod.get_activation_tables = _orig_gat
    _NC = nc
    return nc


def _patch_light_tail():
    """Use sem-only end-of-kernel barriers (the default drain + two full
    all-engine barriers cost ~9us of kernel tail)."""
    from concourse import tile as _tile_mod
    from concourse.vector_clock import ScopedClock

    def _dab_light(self, tick_clock, wait_clock):
        drain_inst = self.nc.sync.drain()
        wait_clock.add_sem_waits(
            drain_inst.ins, ScopedClock({None: tick_clock.global_clock})
        )
        self.nc.all_engine_barrier(sem_only=True)
        popped = self.nc._tile_sem_poison_stack.pop()
        assert popped is self._sem_poison
        self.nc.clear_and_free_semaphores(list(self.sems.allocated().values()))
        self.nc.all_engine_barrier(sem_only=True)

    _tile_mod.TileContext._drain_and_barrier = _dab_light


_patch_light_tail()


def _prep_in_maps(hidden, W, b, trans):
    """Build per-core input dicts (host-side shard/transpose/cast)."""
    wTr = np.ascontiguousarray(
        W.T.reshape(8, 128, L).transpose(1, 0, 2).reshape(128, 8 * L)
    ).astype(BF16)
    texpT = np.exp(trans.astype(np.float64)).astype(np.float32)  # [k, l, r]
    texpT = texpT.transpose(1, 2, 0).reshape(L * L, L)           # [(l r), k]
    texpTr = np.ascontiguousarray(
        texpT.reshape(8, 128, L).transpose(1, 0, 2).reshape(128, 8 * L)
    ).astype(BF16)
    selL, selR = _selectors()

    cpack = np.zeros((128, 1680), dtype=BF16)
    cpack[:, 0:256] = wTr
    cpack[:, 256:512] = texpTr
    cpack[0:L, 512:1536] = selL
    cpack[0:L, 1536:1664] = selR
    cpack[0:L, 1664] = BF16(1.0 / L)
    cpack[0:L, 1665] = b.astype(BF16)

    in_maps = []
    for c in range(N_CORES):
        idx_old = _core_col_heap_index(c)               # old col -> heap row
        # new col -> heap row (pad col maps to row 0, zeroed below)
        rows = np.zeros((COLS, INPUT_SIZE), dtype=BF16)
        real = NEWCOL_TO_OLD < 2047
        rows[real] = hidden[idx_old[NEWCOL_TO_OLD[real]]].astype(BF16)
        m = {"cpack": cpack}
        for g in range(4):
            s = int(BLOCK_STARTS[g])
            blk = rows[s:s + BLOCK_SIZES[g]]            # [ncols, 1024]
            m[f"hsB{g}"] = np.ascontiguousarray(blk.T)  # [1024, ncols]
        in_maps.append(m)
    return in_maps


def _host_finish(results, hidden, W, b, trans):
    """Finish levels 3..10 per core + big-tree top 3 levels, in float64."""
    Texp = np.exp(trans.astype(np.float64)).reshape(L, L * L)   # [k, (l r)]

    score = np.zeros((N_CORES, 256, L))
    elev_nat = {}   # (core, lev) -> [m, L] natural-order E
    for c in range(N_CORES):
        r = results[c]
        E_new = r["outE"].astype(np.float64)            # [L, 2048] new layout
        E = np.empty_like(E_new)
        E[:, NEWCOL_TO_OLD] = E_new                     # back to old layout
        resid2 = r["outResid"].astype(np.float64)       # [L, 256]
        acc2 = r["outAcc"].astype(np.float64)           # [1, 256]
        q = _bitrev(np.arange(256), 8)
        score[c] = (resid2 + acc2)[:, q].T              # node j at col bitrev(j)
        for lev in range(3, SUB_LEVELS):
            m = 1 << (10 - lev)
            qq = _bitrev(np.arange(m), 10 - lev)
            elev_nat[(c, lev)] = E[:, OFFS[lev] + qq].T

    # subtree levels 3..10 (vectorized over cores)
    for lev in range(3, SUB_LEVELS):
        left = score[:, 0::2]
        right = score[:, 1::2]
        Elev = np.stack([elev_nat[(c, lev)] for c in range(N_CORES)])
        ml = left.max(axis=2, keepdims=True)
        mr = right.max(axis=2, keepdims=True)
        P = (np.exp(left - ml)[..., :, None] *
             np.exp(right - mr)[..., None, :]).reshape(N_CORES, -1, L * L)
        score = Elev + np.log(P @ Texp.T) + ml + mr

    # big-tree top: level-3 scores are the 8 subtree roots, heap nodes 7..14
    score = score.reshape(8, L)
    Etop = (hidden[:7].astype(np.float64) @ W.astype(np.float64).T
            + b.astype(np.float64))
    for d in (2, 1, 0):
        left = score[0::2]
        right = score[1::2]
        Elev = Etop[(1 << d) - 1: (1 << (d + 1)) - 1]
        ml = left.max(axis=1, keepdims=True)
        mr = right.max(axis=1, keepdims=True)
        P = (np.exp(left - ml)[:, :, None] *
             np.exp(right - mr)[:, None, :]).reshape(-1, L * L)
        score = Elev + np.log(P @ Texp.T) + ml + mr
    return score[0].astype(np.float32)


def _run_spmd(in_maps, trace=False):
    from concourse.bass_utils import run_bass_kernel_spmd
    nc = _build_bass()
    return run_bass_kernel_spmd(nc, in_maps, list(range(N_CORES)), trace=trace)


def kernel(hidden, W, b, trans):
    hidden = np.asarray(hidden, dtype=np.float32)
    W = np.asarray(W, dtype=np.float32)
    b = np.asarray(b, dtype=np.float32)
    trans = np.asarray(trans, dtype=np.float32)
    in_maps = _prep_in_maps(hidden, W, b, trans)
    res = _run_spmd(in_maps, trace=False)
    return _host_finish(res.results, hidden, W, b, trans)



# revision 3
# speedup vs baseline: 1.4364x; 1.4364x over previous
"""BinaryTreeCRF inside-algorithm kernel for 8 Trainium2 NeuronCores.

Strategy (hardcoded for hidden=[16383,1024], L=32, depth 13):
  - The 16383-node heap tree is cut at big-tree level 3: each of the 8 cores
    owns the 2047-node subtree rooted at heap node 7+c (big levels 3..13).
  - Per core, node hidden states are shipped transposed in fp8-e4m3 (W is
    scaled x64 into fp8; the 1/64 is folded into the bias-add), with tree
    levels bit-reversed so left/right children are contiguous half-blocks,
    and the columns grouped pass-major so each combine pass depends on a
    prefix of the HBM pieces: compute chases the piece loads.
  - The E matmul uses a 4x-replicated stationary operand (M=128 instead of
    M=32, free: the array columns were idle), so E^T lands replicated across
    the 4 partition groups.  That lets the combine selector matmuls run
    row-tiled 4x concurrent (K=32 each, distinct row groups + PSUM banks),
    and the S matmuls use a replicated Texp so resid' is produced replicated
    for the next level with no extra ops.
  - On device (label-on-partition layout, resid/acc decomposition):
      E^T = (W*64) @ hsT / 64 + b      (PE fp8, 8 K-chunks, per piece)
      two combine levels (1024 leaves -> 512 -> 256 nodes):
        logP[(l,r), j] = (resid_l[l,j]-mean_l[j]) + (resid_r[r,j]-mean_r[j])
                         via row-tiled selector matmuls (mean folded in),
        P = exp(logP)  (ACT),  S^T = Texp_rep @ P  (PE),
        resid' = E^T + ln S^T (ACT+DVE), acc' = acc_l+acc_r+mean_l+mean_r.
  - Host finishes the remaining small levels (256 -> subtree roots -> root)
    in float64 numpy; this is ~1% of the FLOPs.
"""

import numpy as np
import ml_dtypes

BF16 = ml_dtypes.bfloat16
FP8 = ml_dtypes.float8_e4m3

INPUT_SIZE = 1024
L = 32
DEPTH = 13
N_CORES = 8
SUB_LEVELS = 11       # per-core subtree levels: 0 = 1024 leaves ... 10 = root
COLS = 2048           # per-core columns (2047 nodes + 1 zero pad)
WSCALE = 64.0         # W is shipped as W*64 in fp8; 1/64 folded into bias-add

# "old" layout: levels from the leaves up, each level bit-reversed.
OFFS = []
_o = 0
for _l in range(SUB_LEVELS):
    OFFS.append(_o)
    _o += 1 << (10 - _l)
assert _o == 2047

# "new" (pass-major) layout: two 768-col blocks [rl | rr | elev] for the two
# level-1 passes (256 parents each), then the level-2 elev block + host tail.
BLOCK_SIZES = [768, 768, 256, 256]
BLOCK_STARTS = np.concatenate([[0], np.cumsum(BLOCK_SIZES)])[:-1]
NEWCOL_TO_OLD = np.empty(COLS, dtype=np.int64)
for _g in range(2):
    _b = _g * 768
    NEWCOL_TO_OLD[_b:_b + 256] = np.arange(_g * 256, _g * 256 + 256)
    NEWCOL_TO_OLD[_b + 256:_b + 512] = 512 + np.arange(_g * 256, _g * 256 + 256)
    NEWCOL_TO_OLD[_b + 512:_b + 768] = 1024 + np.arange(_g * 256, _g * 256 + 256)
NEWCOL_TO_OLD[1536:COLS] = np.arange(1536, COLS)

# HBM pieces (chase granularity): pass 0 = pieces 0-1, pass 1 = pieces 2-3,
# level-2 elev = piece 4, host-tail E = piece 5.
PIECE_SIZES = [384, 384, 384, 384, 256, 256]
PIECE_STARTS = np.concatenate([[0], np.cumsum(PIECE_SIZES)])[:-1]

# cpk_sel columns: [0:1024] selLrep, [1024:1152] selRrep
SEL_COLS = 1152


def _bitrev(x, bits):
    x = np.asarray(x, dtype=np.int64)
    out = np.zeros_like(x)
    for i in range(bits):
        out = (out << 1) | ((x >> i) & 1)
    return out


def _core_col_heap_index(c):
    """heap index for each of the 2047 real old-layout columns of core c."""
    idx = np.zeros(2047, dtype=np.int64)
    for lev in range(SUB_LEVELS):
        m = 1 << (10 - lev)
        d = DEPTH - lev
        q = np.arange(m)
        j = _bitrev(q, 10 - lev)
        idx[OFFS[lev]: OFFS[lev] + m] = (1 << d) - 1 + c * m + j
    return idx


def _selectors():
    """Row-replicated selector matrices (mean-subtraction folded in)."""
    selL = np.full((L, 8 * 128), -1.0 / L, dtype=np.float32)
    selR = np.full((L, 128), -1.0 / L, dtype=np.float32)
    for ch in range(8):
        for p in range(128):
            selL[ch * 4 + p // 32, ch * 128 + p] += 1.0
    for p in range(128):
        selR[p % 32, p] += 1.0
    selLrep = np.tile(selL, (4, 1))          # [128, 1024]
    selRrep = np.tile(selR, (4, 1))          # [128, 128]
    return selLrep.astype(BF16), selRrep.astype(BF16)


_NC = None


def _build_bass():
    global _NC
    if _NC is not None:
        return _NC
    from concourse import bacc, mybir
    from concourse.tile import TileContext

    dtb = mybir.dt.bfloat16
    dtf = mybir.dt.float32
    dt8 = mybir.dt.float8e4
    AF = mybir.ActivationFunctionType
    ALU = mybir.AluOpType

    nc = bacc.Bacc()
    hsP = [nc.dram_tensor(f"hsP{i}", [128, 8 * PIECE_SIZES[i]], dt8,
                          kind="ExternalInput") for i in range(6)]
    cpkS = nc.dram_tensor("cpkS", [128, 2], dtb, kind="ExternalInput")
    wpkD = nc.dram_tensor("wpk", [128, 1024], dt8, kind="ExternalInput")
    cpkL = nc.dram_tensor("cpkL", [128, SEL_COLS], dtb, kind="ExternalInput")
    cpkT = nc.dram_tensor("cpkT", [128, 1024], dtb, kind="ExternalInput")
    outE = nc.dram_tensor("outE", [L, COLS], dtb, kind="ExternalOutput")
    outResid = nc.dram_tensor("outResid", [L, 256], dtb, kind="ExternalOutput")
    outAcc = nc.dram_tensor("outAcc", [1, 256], dtf, kind="ExternalOutput")

    with TileContext(nc) as tc:
        with tc.tile_pool(name="consts", bufs=1) as consts, \
             tc.tile_pool(name="hs", bufs=1) as hpool, \
             tc.tile_pool(name="state", bufs=1) as state, \
             tc.tile_pool(name="pbuf", bufs=2) as pbuf, \
             tc.tile_pool(name="tmp", bufs=4) as tmp, \
             tc.tile_pool(name="psE", bufs=2, space="PSUM") as psE_pool, \
             tc.tile_pool(name="logP", bufs=2, space="PSUM") as logP_pool, \
             tc.tile_pool(name="smean", bufs=2, space="PSUM") as smean_pool:

            # --- PE warm-up: junk matmuls with NO DMA dependency (memset
            # tile) keep the HAM busy through preamble + first loads.
            junk = state.tile([128, 512], dtb, tag="junk")
            nc.gpsimd.memset(junk, 0.0)
            for _ in range(8):
                jp = smean_pool.tile([128, 2, 256], dtf, tag="sm")
                nc.tensor.matmul(jp[:, :, :], lhsT=junk[:, 0:128],
                                 rhs=junk, start=True, stop=True)

            # --- const + hidden loads (2 HWDGE lanes, FIFO per lane)
            cS = consts.tile([128, 2], dtb, tag="cS")
            nc.sync.dma_start(out=cS, in_=cpkS[:, :])
            wp = consts.tile([128, 1024], dt8, tag="wp")
            nc.scalar.dma_start(out=wp, in_=wpkD[:, :])

            hst = []
            for i in range(6):
                t = hpool.tile([128, 8, PIECE_SIZES[i]], dt8, tag=f"hs{i}")
                hst.append(t)
            cL = consts.tile([128, SEL_COLS], dtb, tag="cL")
            cT = consts.tile([128, 1024], dtb, tag="cT")
            # sync lane: hsP0, texp, hsP2, hsP4 ; scalar: hsP1, sel, hsP3, hsP5
            nc.sync.dma_start(out=hst[0],
                              in_=hsP[0][:, :].rearrange("p (c n) -> p c n", c=8))
            nc.scalar.dma_start(out=hst[1],
                                in_=hsP[1][:, :].rearrange("p (c n) -> p c n", c=8))
            nc.scalar.dma_start(out=cL, in_=cpkL[:, :])
            nc.sync.dma_start(out=cT, in_=cpkT[:, :])
            nc.sync.dma_start(out=hst[2],
                              in_=hsP[2][:, :].rearrange("p (c n) -> p c n", c=8))
            nc.scalar.dma_start(out=hst[3],
                                in_=hsP[3][:, :].rearrange("p (c n) -> p c n", c=8))
            nc.sync.dma_start(out=hst[4],
                              in_=hsP[4][:, :].rearrange("p (c n) -> p c n", c=8))
            nc.scalar.dma_start(out=hst[5],
                                in_=hsP[5][:, :].rearrange("p (c n) -> p c n", c=8))

            ones128_t = cS[:, 0:1]
            bias_t = cS[:, 1:2]
            selL_t = cL[:, 0:1024]
            selR_t = cL[:, 1024:1152]

            # Upcast bias to f32 (tensor_scalar needs an f32 scalar AP).
            bias_f = tmp.tile([128, 1], dtf, tag="bias_f")
            nc.scalar.activation(out=bias_f, in_=bias_t, func=AF.Identity)

            E_bf = state.tile([128, COLS], dtb, tag="E_bf")
            resid1 = state.tile([128, 512], dtb, tag="resid1")
            acc1 = state.tile([1, 512], dtf, tag="acc1")
            resid2 = state.tile([128, 256], dtb, tag="resid2")
            acc2 = state.tile([1, 256], dtf, tag="acc2")

            def emit_E_piece(i):
                s = int(PIECE_STARTS[i])
                n = PIECE_SIZES[i]
                ps = psE_pool.tile([128, 384], dtf, tag="ps")
                for c in range(8):
                    nc.tensor.matmul(
                        ps[:, 0:n], lhsT=wp[:, c * 128:(c + 1) * 128],
                        rhs=hst[i][:, c, :],
                        start=(c == 0), stop=(c == 7))
                # E = psE/64 + bias  (W was shipped x64 in fp8)
                nc.vector.tensor_scalar(
                    out=E_bf[:, s:s + n], in0=ps[:, 0:n],
                    scalar1=1.0 / WSCALE, scalar2=bias_f,
                    op0=ALU.mult, op1=ALU.add)
                # early E writeback (alternate HWDGE lanes)
                eng = nc.sync if i % 2 == 0 else nc.scalar
                eng.dma_start(out=outE[:, s:s + n], in_=E_bf[0:L, s:s + n])

            def combine_pass(rl, rr, elev, r_out, acc_out_fn):
                """One combine pass over 256 parents (replicated operands).

                rl/rr/elev are [128, 256] replicated slices; r_out is a
                [128, 256] replicated resid output slice."""
                logPa = logP_pool.tile([128, 4, 256], dtf, tag="lp")
                logPb = logP_pool.tile([128, 4, 256], dtf, tag="lp")
                # wave A: chunks 0,2,4,6 at row groups 0-3, distinct banks;
                # wave B: chunks 1,3,5,7.
                for c in (0, 2, 4, 6, 1, 3, 5, 7):
                    g = (c // 2) % 4 if c % 2 == 0 else ((c - 1) // 2) % 4
                    tile = logPa if c < 4 else logPb
                    lp = tile[:, c % 4, :]
                    nc.tensor.matmul(
                        lp, lhsT=selL_t[32 * g:32 * (g + 1), c * 128:(c + 1) * 128],
                        rhs=rl[32 * g:32 * (g + 1), :], start=True, stop=False,
                        tile_position=(32 * g, 0))
                    nc.tensor.matmul(
                        lp, lhsT=selR_t[32 * g:32 * (g + 1), :],
                        rhs=rr[32 * g:32 * (g + 1), :], start=False, stop=True,
                        tile_position=(32 * g, 0))
                # meansum = mean_l + mean_r via K=128 ones/128 matmul
                sm = smean_pool.tile([128, 2, 256], dtf, tag="sm")
                mean = sm[0:1, 1, :]
                nc.tensor.matmul(mean, lhsT=ones128_t, rhs=rl,
                                 start=True, stop=False)
                nc.tensor.matmul(mean, lhsT=ones128_t, rhs=rr,
                                 start=False, stop=True)
                P = pbuf.tile([128, 8, 256], dtb, tag="P")
                nc.scalar.activation(out=P[:, 0:4, :], in_=logPa, func=AF.Exp)
                nc.scalar.activation(out=P[:, 4:8, :], in_=logPb, func=AF.Exp)
                S = sm[:, 0, :]
                for c in range(8):
                    nc.tensor.matmul(
                        S, lhsT=cT[:, c * 128:(c + 1) * 128], rhs=P[:, c, :],
                        start=(c == 0), stop=(c == 7))
                lnS = tmp.tile([128, 256], dtb, tag="lnS")
                nc.scalar.activation(out=lnS, in_=S, func=AF.Ln)
                nc.vector.tensor_add(r_out, lnS, elev)
                acc_out_fn(mean)

            # level-1 passes chase their pieces
            for g in range(2):
                emit_E_piece(2 * g)
                emit_E_piece(2 * g + 1)
                b = g * 768
                combine_pass(
                    rl=E_bf[:, b:b + 256],
                    rr=E_bf[:, b + 256:b + 512],
                    elev=E_bf[:, b + 512:b + 768],
                    r_out=resid1[:, g * 256:(g + 1) * 256],
                    acc_out_fn=lambda mean, g=g: nc.vector.tensor_copy(
                        acc1[:, g * 256:(g + 1) * 256], mean))

            emit_E_piece(4)   # level-2 elev columns
            emit_E_piece(5)   # host-tail columns

            # level 2: 512 -> 256, one pass of 256 parents
            usum = tmp.tile([1, 256], dtf, tag="usum")

            def acc2_fn(mean):
                nc.vector.tensor_add(usum, acc1[:, 0:256], acc1[:, 256:512])
                nc.vector.tensor_add(acc2, usum, mean)

            combine_pass(
                rl=resid1[:, 0:256],
                rr=resid1[:, 256:512],
                elev=E_bf[:, 1536:1792],
                r_out=resid2,
                acc_out_fn=acc2_fn)

            nc.sync.dma_start(out=outResid[:, :], in_=resid2[0:L, :])
            nc.scalar.dma_start(out=outAcc[:, :], in_=acc2)

    # Pin Exp/Ln/Identity to the one table set containing all three, so the
    # ACT engine loads its function table exactly once.
    import concourse.bacc as _bacc_mod
    from concourse.hw_specs import get_activation_tables as _gat
    _keep = "natural_log_exp_and_others"
    _pin = {AF.Exp, AF.Ln, AF.Identity, AF.Copy}

    def _gat_pinned(arch):
        t = _gat(arch)
        return {name: (funcs if name == _keep else (set(funcs) - _pin))
                for name, funcs in t.items()}

    _orig_gat = _bacc_mod.get_activation_tables
    _bacc_mod.get_activation_tables = _gat_pinned
    try:
        nc.compile()
    finally:
        _bacc_mod.get_activation_tables = _orig_gat
    _NC = nc
    return nc


def _patch_light_tail():
    """Use sem-only end-of-kernel barriers (the default drain + two full
    all-engine barriers cost ~9us of kernel tail)."""
    from concourse import tile as _tile_mod
    from concourse.vector_clock import ScopedClock

    def _dab_light(self, tick_clock, wait_clock):
        drain_inst = self.nc.sync.drain()
        wait_clock.add_sem_waits(
            drain_inst.ins, ScopedClock({None: tick_clock.global_clock})
        )
        self.nc.all_engine_barrier(sem_only=True)
        popped = self.nc._tile_sem_poison_stack.pop()
        assert popped is self._sem_poison
        self.nc.clear_and_free_semaphores(list(self.sems.allocated().values()))
        self.nc.all_engine_barrier(sem_only=True)

    _tile_mod.TileContext._drain_and_barrier = _dab_light


_patch_light_tail()


def _prep_in_maps(hidden, W, b, trans):
    """Build per-core input dicts (host-side shard/transpose/cast)."""
    # wpk[p, c*128 + 32g + m] = 64*W[m, c*128+p], fp8
    WT = W.T.reshape(8, 128, L)                       # [c, p, m]
    wpk_chunk = (WT * WSCALE).astype(FP8)             # [c, p, m] fp8
    wpk = np.zeros((128, 1024), dtype=FP8)
    for c in range(8):
        for g in range(4):
            wpk[:, c * 128 + 32 * g: c * 128 + 32 * (g + 1)] = wpk_chunk[c]

    texpT = np.exp(trans.astype(np.float64)).astype(np.float32)  # [k, l, r]
    texpT = texpT.transpose(1, 2, 0).reshape(L * L, L)           # [(l r), k]
    # cpkT[p, c*128 + 32g + k] = texpT[c*128+p, k]
    cpkT = np.zeros((128, 1024), dtype=BF16)
    for c in range(8):
        blk = texpT[c * 128:(c + 1) * 128].astype(BF16)          # [128, 32]
        for g in range(4):
            cpkT[:, c * 128 + 32 * g: c * 128 + 32 * (g + 1)] = blk

    selLrep, selRrep = _selectors()
    cpkL = np.zeros((128, SEL_COLS), dtype=BF16)
    cpkL[:, 0:1024] = selLrep
    cpkL[:, 1024:1152] = selRrep

    cpkS = np.zeros((128, 2), dtype=BF16)
    cpkS[:, 0] = BF16(1.0 / 128.0)
    cpkS[:, 1] = np.tile(b.astype(BF16), 4)

    in_maps = []
    for c in range(N_CORES):
        idx_old = _core_col_heap_index(c)               # old col -> heap row
        rows = np.zeros((COLS, INPUT_SIZE), dtype=FP8)
        real = NEWCOL_TO_OLD < 2047
        rows[real] = hidden[idx_old[NEWCOL_TO_OLD[real]]].astype(FP8)
        m = {"cpkS": cpkS, "wpk": wpk, "cpkL": cpkL, "cpkT": cpkT}
        for i in range(6):
            s = int(PIECE_STARTS[i])
            n = PIECE_SIZES[i]
            blk = rows[s:s + n]                         # [n, 1024]
            # hsP[p, c*n + j] = hidden[node j, c*128+p]
            piece = np.ascontiguousarray(
                blk.T.reshape(8, 128, n).transpose(1, 0, 2).reshape(128, 8 * n))
            m[f"hsP{i}"] = piece
        in_maps.append(m)
    return in_maps


def _host_finish(results, hidden, W, b, trans):
    """Finish levels 3..10 per core + big-tree top 3 levels, in float64."""
    Texp = np.exp(trans.astype(np.float64)).reshape(L, L * L)   # [k, (l r)]

    score = np.zeros((N_CORES, 256, L))
    elev_nat = {}   # (core, lev) -> [m, L] natural-order E
    for c in range(N_CORES):
        r = results[c]
        E_new = r["outE"].astype(np.float64)            # [L, 2048] new layout
        E = np.empty_like(E_new)
        E[:, NEWCOL_TO_OLD] = E_new                     # back to old layout
        resid2 = r["outResid"].astype(np.float64)       # [L, 256]
        acc2 = r["outAcc"].astype(np.float64)           # [1, 256]
        q = _bitrev(np.arange(256), 8)
        score[c] = (resid2 + acc2)[:, q].T              # node j at col bitrev(j)
        for lev in range(3, SUB_LEVELS):
            m = 1 << (10 - lev)
            qq = _bitrev(np.arange(m), 10 - lev)
            elev_nat[(c, lev)] = E[:, OFFS[lev] + qq].T

    # subtree levels 3..10 (vectorized over cores)
    for lev in range(3, SUB_LEVELS):
        left = score[:, 0::2]
        right = score[:, 1::2]
        Elev = np.stack([elev_nat[(c, lev)] for c in range(N_CORES)])
        ml = left.max(axis=2, keepdims=True)
        mr = right.max(axis=2, keepdims=True)
        P = (np.exp(left - ml)[..., :, None] *
             np.exp(right - mr)[..., None, :]).reshape(N_CORES, -1, L * L)
        score = Elev + np.log(P @ Texp.T) + ml + mr

    # big-tree top: level-3 scores are the 8 subtree roots, heap nodes 7..14
    score = score.reshape(8, L)
    Etop = (hidden[:7].astype(np.float64) @ W.astype(np.float64).T
            + b.astype(np.float64))
    for d in (2, 1, 0):
        left = score[0::2]
        right = score[1::2]
        Elev = Etop[(1 << d) - 1: (1 << (d + 1)) - 1]
        ml = left.max(axis=1, keepdims=True)
        mr = right.max(axis=1, keepdims=True)
        P = (np.exp(left - ml)[:, :, None] *
             np.exp(right - mr)[:, None, :]).reshape(-1, L * L)
        score = Elev + np.log(P @ Texp.T) + ml + mr
    return score[0].astype(np.float32)


def _run_spmd(in_maps, trace=False):
    from concourse.bass_utils import run_bass_kernel_spmd
    nc = _build_bass()
    return run_bass_kernel_spmd(nc, in_maps, list(range(N_CORES)), trace=trace)


def kernel(hidden, W, b, trans):
    hidden = np.asarray(hidden, dtype=np.float32)
    W = np.asarray(W, dtype=np.float32)
    b = np.asarray(b, dtype=np.float32)
    trans = np.asarray(trans, dtype=np.float32)
    in_maps = _prep_in_maps(hidden, W, b, trans)
    res = _run_spmd(in_maps, trace=False)
    return _host_finish(res.results, hidden, W, b, trans)
